# revision 4
# baseline (speedup 1.0000x reference)
"""CircularMemoryBank on 8 trn2 NeuronCores — single launch, staged For_i loops.

Math (D = 4096):
  store:    m[d]   = sum_i sum_j K[i,j] * V[i, (d-j) mod D]
  retrieve: R[q,n] = sum_b Q[q,b] * m[(b+n) mod D]

One kernel launch does everything on-device:
  1. store: H[r, 512b+x] = sum_c sum_i K[i,128c+r] V[i,(512b+x-128c)%D].
     For_i over the 8 output banks b; each iteration DMA-stages the V
     window span [512b+128, +4480) per i-chunk from a doubled-V DRAM
     buffer (one symbolic offset per DMA, on the DMA queue's registers),
     then runs 128 fully-static accumulating matmuls (PE weights need
     static offsets; register-backed offsets are capped ~12/engine, so
     staging keeps PE at zero symbolic operands).
  2. skew-reduce: m_c[d] = sum_r H[r,(d-r)%D] via a DRAM-bounced diagonal
     DMA (H|H in DRAM, read back with row stride 2D-1) + 32 ones-matmuls
     reducing across partitions.
  3. AllReduce(sum) of the 16KB partial m across the 8 cores.
  4. retrieve: R[q,n] = sum_bc Q^T-tile(bc,qb) x m-window. For_i over the
     8 n-windows w; each iteration stages call[p, z] = m[(p+512w+z)%D]
     with one overlapping-window DMA from the doubled-m DRAM buffer, then
     4 q-blocks x 32 static matmuls accumulate R in natural [q, n] layout.
     Q^T comes from 32 on-device transpose DMAs that overlap the store.

Output r_out is bf16 [512, 4096] per core; per-core outputs concatenate
directly into the full R (cast to f32 on host).  Import-time background
threads warm the axon/jax session and build the Bass program (both are
input-independent).
"""

import threading
import numpy as np
import ml_dtypes

import concourse.bass as bass
import concourse.mybir as mybir
import concourse.tile as tile
from concourse.bass_types import AP
from concourse.bass_utils import run_bass_kernel_spmd

D = 4096
NCORES = 8
NS = D // NCORES  # 512 rows per core
NI = NS // 128    # 4 i-chunks
BF16 = mybir.dt.bfloat16
F32 = mybir.dt.float32
NPBF16 = ml_dtypes.bfloat16

LAST_EXEC_NS = []  # wall-clock ns per launch

_ws_ctr = [0]


def _split_waits(nc, cap=1):
    """walrus ISA structs hold very few sem-wait slots (1 for Matmult).

    Hoist excess waits from any instruction onto freshly inserted same-engine
    NoOps placed immediately before it, one wait per NoOp.
    """
    for f in nc.m.functions:
        for bb in f.blocks:
            insts = bb.instructions
            out = []
            changed = False
            for ins in insts:
                si = ins.sync_info() if callable(ins.sync_info) else \
                    ins.sync_info
                if si is not None and len(si.on_wait) > cap:
                    waits = list(si.on_wait)
                    for w in waits[:-cap]:
                        nop = mybir.InstNoOp(name=f"ws_{_ws_ctr[0]}")
                        _ws_ctr[0] += 1
                        nop.engine = ins.engine
                        nop.sync_info = mybir.SyncInfo(on_wait=[w],
                                                       on_update=[])
                        out.append(nop)
                    ins.sync_info = mybir.SyncInfo(
                        on_wait=waits[-cap:], on_update=list(si.on_update))
                    changed = True
                out.append(ins)
            if changed:
                bb.instructions = out
    return nc


STG = 4480  # staged V window width: c spans 32*128=4096, +512 window, -128


def _build():
    nc = bass.Bass("TRN2", target_bir_lowering=False, debug=False,
                   num_devices=NCORES)
    k_in = nc.dram_tensor("k_in", [NS, D], BF16, kind="ExternalInput")
    v_in = nc.dram_tensor("v_in", [NS, D], BF16, kind="ExternalInput")
    q_in = nc.dram_tensor("q_in", [NS, D], BF16, kind="ExternalInput")
    r_out = nc.dram_tensor("r_out", [NS, D], BF16, kind="ExternalOutput")

    vcat_d = nc.dram_tensor("vcat_d", [128, NI * 2 * D], BF16,
                            kind="Internal")
    hcat_d = nc.dram_tensor("hcat_d", [128, 2 * D], F32, kind="Internal")
    m_part = nc.dram_tensor("m_part", [1, D], F32, kind="Internal")
    m_red = nc.dram_tensor("m_red", [1, D], F32, kind="Internal")
    mcat = nc.dram_tensor("mcat", [1, 2 * D + 128], BF16, kind="Internal")

    with tile.TileContext(nc) as tc:
        with (
            tc.tile_pool(name="sb", bufs=1) as sb,
            tc.tile_pool(name="ps", bufs=1, space="PSUM") as ps,
        ):
            # ---- input loads ----
            k_all = sb.tile([128, NI * D], BF16, name="k_all")
            nc.sync.dma_start(
                k_all[:].rearrange("p (i j) -> p i j", i=NI),
                k_in.rearrange("(i p) j -> p i j", p=128))

            # doubled V per i-chunk in DRAM: vcat_d[p, 2D*i + D*t + j]
            # = V[128i + p, j]
            for i in range(NI):
                for t in range(2):
                    nc.sync.dma_start(
                        vcat_d[:, 2 * D * i + D * t:2 * D * i + D * (t + 1)],
                        v_in[128 * i:128 * (i + 1), :])

            # qt_all[p, 512*bc + q] = Q[q, 128*bc + p]; overlaps store
            qt_all = sb.tile([128, 32 * NS], BF16, name="qt_all")
            for bc in range(32):
                nc.sync.dma_start(
                    qt_all[:, NS * bc:NS * (bc + 1)],
                    q_in[0:NS, 128 * bc:128 * (bc + 1)].transpose([1, 0]))

            # ---- store: For_i over banks, staged V windows ----
            h_sb = sb.tile([128, D], F32, name="h_sb")
            stg = [sb.tile([128, STG], BF16, name=f"stg{i}")
                   for i in range(NI)]
            with tc.For_i(0, 8) as bv:
                for i in range(NI):
                    # stage vcat_d[p, 2D*i + 512b + 128 + y], y in [0, STG)
                    nc.sync.dma_start(
                        stg[i][:],
                        vcat_d[:, bass.ds(bv * 512 + (2 * D * i + 128),
                                          STG)])
                h_ps = ps.tile([128, 512], F32, name="h_ps")
                for c in range(32):
                    for i in range(NI):
                        # moving = vcat_d[.., 2D*i + D + 512b - 128c ..+512]
                        #        = stg[i][:, D - 128 - 128c .. +512]
                        o = D - 128 - 128 * c
                        nc.tensor.matmul(
                            h_ps[:],
                            k_all[:, D * i + 128 * c:D * i + 128 * (c + 1)],
                            stg[i][:, o:o + 512],
                            start=(c == 0 and i == 0),
                            stop=(c == 31 and i == NI - 1),
                        )
                nc.vector.tensor_copy(h_sb[:, bass.ds(bv * 512, 512)],
                                      h_ps[:])

            # ---- skew-reduce: m[d] = sum_r H[r, (d-r) % D] ----
            nc.sync.dma_start(hcat_d[:, 0:D], h_sb[:])
            nc.sync.dma_start(hcat_d[:, D:2 * D], h_sb[:])
            hs = sb.tile([128, D], F32, name="hs")
            nc.sync.dma_start(
                hs[:], AP(hcat_d[:].tensor, D, [[2 * D - 1, 128], [1, D]]))

            ones = sb.tile([128, 1], F32, name="ones")
            nc.vector.memset(ones[:], 1.0)
            mc_ps = ps.tile([128, 32], F32, name="mc_ps")
            for t in range(32):
                nc.tensor.matmul(
                    mc_ps[:, t:t + 1],
                    hs[:, 128 * t:128 * (t + 1)],
                    ones[:],
                    start=(t == 0), stop=(t == 31),
                )
            mcol_sb = sb.tile([128, 32], F32, name="mcol_sb")
            nc.vector.tensor_copy(mcol_sb[:], mc_ps[:])
            # m_part[128t + p] = mcol_sb[p, t]
            nc.sync.dma_start(
                AP(m_part[:].tensor, 0, [[1, 128], [128, 32]]), mcol_sb[:])

            # ---- AllReduce the 16KB partial m ----
            nc.gpsimd.collective_compute(
                "AllReduce", mybir.AluOpType.add,
                replica_groups=[[i for i in range(NCORES)]],
                ins=[m_part[:]], outs=[m_red[:]],
            )

            # ---- doubled m in DRAM (bf16) ----
            m_sb = sb.tile([1, D], F32, name="m_sb")
            nc.sync.dma_start(m_sb[:], m_red[:])
            m_bf = sb.tile([1, D], BF16, name="m_bf")
            nc.vector.tensor_copy(m_bf[:], m_sb[:])
            nc.sync.dma_start(mcat[:, 0:D], m_bf[:])
            nc.sync.dma_start(mcat[:, D:2 * D], m_bf[:])
            nc.sync.dma_start(mcat[:, 2 * D:2 * D + 128], m_bf[:, 0:128])

            # ---- retrieve: For_i over windows, staged circulant ----
            r_all = sb.tile([128, NI * D], BF16, name="r_all")
            stgr = sb.tile([128, STG], BF16, name="stgr")
            with tc.For_i(0, 8) as wv:
                # stgr[p, z] = mcat[p + 512w + z], z in [0, 4480)
                nc.sync.dma_start(
                    stgr[:],
                    AP(mcat[:].tensor, wv * 512, [[1, 128], [1, STG]]))
                for qb in range(4):
                    r_ps = ps.tile([128, 512], F32, name=f"r_ps{qb}")
                    for bc in range(32):
                        nc.tensor.matmul(
                            r_ps[:],
                            qt_all[:, NS * bc + 128 * qb:
                                   NS * bc + 128 * qb + 128],
                            stgr[:, 128 * bc:128 * bc + 512],
                            start=(bc == 0), stop=(bc == 31),
                        )
                    dst = bass.ds(wv * 512 + D * qb, 512)
                    if qb % 2 == 0:
                        nc.vector.tensor_copy(r_all[:, dst], r_ps[:])
                    else:
                        nc.scalar.copy(r_all[:, dst], r_ps[:])
            nc.sync.dma_start(
                r_out.rearrange("(qb p) n -> p qb n", p=128),
                r_all[:].rearrange("p (qb n) -> p qb n", qb=NI))
    _split_waits(nc)
    return nc


_warm = {"nc": None, "err": None}


def _warm_jax():
    try:
        import jax
        jax.devices()
    except Exception:
        pass


def _warm_build():
    try:
        _warm["nc"] = _build()
    except Exception as e:  # rebuild inline in kernel() on failure
        _warm["err"] = e


_thr_jax = threading.Thread(target=_warm_jax, daemon=True)
_thr_jax.start()
_thr_build = threading.Thread(target=_warm_build, daemon=True)
_thr_build.start()


def _run(nc, in_maps):
    import time
    t0 = time.time()
    res = run_bass_kernel_spmd(nc, in_maps, core_ids=list(range(NCORES)))
    LAST_EXEC_NS.append(int((time.time() - t0) * 1e9))
    return res.results


def kernel(keys, values, query_keys):
    keys = np.asarray(keys)
    values = np.asarray(values)
    query_keys = np.asarray(query_keys)

    in_maps = []
    for c in range(NCORES):
        sl = slice(NS * c, NS * (c + 1))
        in_maps.append({
            "k_in": np.ascontiguousarray(keys[sl].astype(NPBF16)),
            "v_in": np.ascontiguousarray(values[sl].astype(NPBF16)),
            "q_in": np.ascontiguousarray(query_keys[sl].astype(NPBF16)),
        })

    _thr_jax.join(timeout=300)
    _thr_build.join(timeout=300)
    nc = _warm["nc"] if _warm["nc"] is not None else _build()

    # The terminal occasionally reports NRT_EXEC_UNIT_UNRECOVERABLE and
    # needs ~1-2 min to self-heal; retry rather than fail the whole call.
    outs = None
    for attempt, delay in ((0, 0), (1, 30), (2, 120)):
        try:
            if attempt:
                import time
                time.sleep(delay)
                try:
                    import jax
                    jax.clear_caches()
                except Exception:
                    pass
                nc = _build()
            outs = _run(nc, in_maps)
            break
        except Exception:
            if attempt == 2:
                raise
    return np.concatenate(
        [outs[c]["r_out"] for c in range(NCORES)], axis=0).astype(np.float32)


# revision 5
# speedup vs baseline: 1.0531x; 1.0531x over previous
"""CircularMemoryBank on 8 trn2 NeuronCores — single launch, staged For_i loops.

Math (D = 4096):
  store:    m[d]   = sum_i sum_j K[i,j] * V[i, (d-j) mod D]
  retrieve: R[q,n] = sum_b Q[q,b] * m[(b+n) mod D]

One kernel launch does everything on-device:
  1. store: H[r, 512b+x] = sum_c sum_i K[i,128c+r] V[i,(512b+x-128c)%D].
     For_i over the 8 output banks b; each iteration DMA-stages the V
     window span [512b+128, +4480) per i-chunk from a doubled-V DRAM
     buffer (one symbolic offset per DMA, on the DMA queue's registers),
     then runs 128 fully-static accumulating matmuls (PE weights need
     static offsets; register-backed offsets are capped ~12/engine, so
     staging keeps PE at zero symbolic operands).
  2. skew-reduce: m_c[d] = sum_r H[r,(d-r)%D] via a DRAM-bounced diagonal
     DMA (H|H in DRAM, read back with row stride 2D-1) + 32 ones-matmuls
     reducing across partitions.
  3. AllReduce(sum) of the 16KB partial m across the 8 cores.
  4. retrieve: R[q,n] = sum_bc Q^T-tile(bc,qb) x m-window. For_i over the
     8 n-windows w; each iteration stages call[p, z] = m[(p+512w+z)%D]
     with one overlapping-window DMA from the doubled-m DRAM buffer, then
     4 q-blocks x 32 static matmuls accumulate R in natural [q, n] layout.
     Q^T comes from 32 on-device transpose DMAs that overlap the store.

Output r_out is bf16 [512, 4096] per core; per-core outputs concatenate
directly into the full R (cast to f32 on host).  Import-time background
threads warm the axon/jax session and build the Bass program (both are
input-independent).
"""

import threading
import numpy as np
import ml_dtypes

import concourse.bass as bass
import concourse.mybir as mybir
import concourse.tile as tile
from concourse.bass_types import AP
from concourse.bass_utils import run_bass_kernel_spmd

D = 4096
NCORES = 8
NS = D // NCORES  # 512 rows per core
NI = NS // 128    # 4 i-chunks
BF16 = mybir.dt.bfloat16
F32 = mybir.dt.float32
NPBF16 = ml_dtypes.bfloat16

LAST_EXEC_NS = []  # wall-clock ns per launch

_ws_ctr = [0]


def _split_waits(nc, cap=1):
    """walrus ISA structs hold very few sem-wait slots (1 for Matmult).

    Hoist excess waits from any instruction onto freshly inserted same-engine
    NoOps placed immediately before it, one wait per NoOp.
    """
    for f in nc.m.functions:
        for bb in f.blocks:
            insts = bb.instructions
            out = []
            changed = False
            for ins in insts:
                si = ins.sync_info() if callable(ins.sync_info) else \
                    ins.sync_info
                if si is not None and len(si.on_wait) > cap:
                    waits = list(si.on_wait)
                    for w in waits[:-cap]:
                        nop = mybir.InstNoOp(name=f"ws_{_ws_ctr[0]}")
                        _ws_ctr[0] += 1
                        nop.engine = ins.engine
                        nop.sync_info = mybir.SyncInfo(on_wait=[w],
                                                       on_update=[])
                        out.append(nop)
                    ins.sync_info = mybir.SyncInfo(
                        on_wait=waits[-cap:], on_update=list(si.on_update))
                    changed = True
                out.append(ins)
            if changed:
                bb.instructions = out
    return nc


STG = 4480  # staged V window width: c spans 32*128=4096, +512 window, -128


def _build():
    nc = bass.Bass("TRN2", target_bir_lowering=False, debug=False,
                   num_devices=NCORES)
    k_in = nc.dram_tensor("k_in", [NS, D], BF16, kind="ExternalInput")
    v_in = nc.dram_tensor("v_in", [NS, D], BF16, kind="ExternalInput")
    q_in = nc.dram_tensor("q_in", [NS, D], BF16, kind="ExternalInput")
    r_out = nc.dram_tensor("r_out", [NS, D], BF16, kind="ExternalOutput")

    vcat_d = nc.dram_tensor("vcat_d", [128, NI * 2 * D], BF16,
                            kind="Internal")
    hcat_d = nc.dram_tensor("hcat_d", [128, 2 * D], F32, kind="Internal")
    m_part = nc.dram_tensor("m_part", [1, D], F32, kind="Internal")
    m_red = nc.dram_tensor("m_red", [1, D], F32, kind="Internal")
    mcat = nc.dram_tensor("mcat", [1, 2 * D + 128], BF16, kind="Internal")

    with tile.TileContext(nc) as tc:
        with (
            tc.tile_pool(name="sb", bufs=1) as sb,
            tc.tile_pool(name="ps", bufs=1, space="PSUM") as ps,
        ):
            # ---- input loads ----
            k_all = sb.tile([128, NI * D], BF16, name="k_all")
            nc.sync.dma_start(
                k_all[:].rearrange("p (i j) -> p i j", i=NI),
                k_in.rearrange("(i p) j -> p i j", p=128))

            # doubled V per i-chunk in DRAM: vcat_d[p, 2D*i + D*t + j]
            # = V[128i + p, j]
            for i in range(NI):
                for t in range(2):
                    nc.sync.dma_start(
                        vcat_d[:, 2 * D * i + D * t:2 * D * i + D * (t + 1)],
                        v_in[128 * i:128 * (i + 1), :])

            # qt_all[p, 512*bc + q] = Q[q, 128*bc + p]; overlaps store
            qt_all = sb.tile([128, 32 * NS], BF16, name="qt_all")
            for bc in range(32):
                nc.sync.dma_start(
                    qt_all[:, NS * bc:NS * (bc + 1)],
                    q_in[0:NS, 128 * bc:128 * (bc + 1)].transpose([1, 0]))

            # ---- store: For_i over banks, staged V windows ----
            h_sb = sb.tile([128, D], F32, name="h_sb")
            stg = [sb.tile([128, STG], BF16, name=f"stg{i}")
                   for i in range(NI)]
            with tc.For_i(0, 8) as bv:
                for i in range(NI):
                    # stage vcat_d[p, 2D*i + 512b + 128 + y], y in [0, STG)
                    nc.sync.dma_start(
                        stg[i][:],
                        vcat_d[:, bass.ds(bv * 512 + (2 * D * i + 128),
                                          STG)])
                h_ps = ps.tile([128, 512], F32, name="h_ps")
                for c in range(32):
                    for i in range(NI):
                        # moving = vcat_d[.., 2D*i + D + 512b - 128c ..+512]
                        #        = stg[i][:, D - 128 - 128c .. +512]
                        o = D - 128 - 128 * c
                        nc.tensor.matmul(
                            h_ps[:],
                            k_all[:, D * i + 128 * c:D * i + 128 * (c + 1)],
                            stg[i][:, o:o + 512],
                            start=(c == 0 and i == 0),
                            stop=(c == 31 and i == NI - 1),
                        )
                nc.vector.tensor_copy(h_sb[:, bass.ds(bv * 512, 512)],
                                      h_ps[:])

            # ---- skew-reduce: m[d] = sum_r H[r, (d-r) % D] ----
            nc.sync.dma_start(hcat_d[:, 0:D], h_sb[:])
            nc.sync.dma_start(hcat_d[:, D:2 * D], h_sb[:])
            hs = sb.tile([128, D], F32, name="hs")
            nc.sync.dma_start(
                hs[:], AP(hcat_d[:].tensor, D, [[2 * D - 1, 128], [1, D]]))

            ones = sb.tile([128, 1], F32, name="ones")
            nc.vector.memset(ones[:], 1.0)
            mc_ps = ps.tile([128, 32], F32, name="mc_ps")
            for t in range(32):
                nc.tensor.matmul(
                    mc_ps[:, t:t + 1],
                    hs[:, 128 * t:128 * (t + 1)],
                    ones[:],
                    start=(t == 0), stop=(t == 31),
                )
            mcol_sb = sb.tile([128, 32], F32, name="mcol_sb")
            nc.vector.tensor_copy(mcol_sb[:], mc_ps[:])
            # m_part[128t + p] = mcol_sb[p, t]
            nc.sync.dma_start(
                AP(m_part[:].tensor, 0, [[1, 128], [128, 32]]), mcol_sb[:])

            # ---- AllReduce the 16KB partial m ----
            nc.gpsimd.collective_compute(
                "AllReduce", mybir.AluOpType.add,
                replica_groups=[[i for i in range(NCORES)]],
                ins=[m_part[:]], outs=[m_red[:]],
            )

            # ---- doubled m in DRAM (bf16) ----
            m_sb = sb.tile([1, D], F32, name="m_sb")
            nc.sync.dma_start(m_sb[:], m_red[:])
            m_bf = sb.tile([1, D], BF16, name="m_bf")
            nc.vector.tensor_copy(m_bf[:], m_sb[:])
            nc.sync.dma_start(mcat[:, 0:D], m_bf[:])
            nc.sync.dma_start(mcat[:, D:2 * D], m_bf[:])
            nc.sync.dma_start(mcat[:, 2 * D:2 * D + 128], m_bf[:, 0:128])

            # ---- retrieve: For_i over windows, staged circulant ----
            r_all = sb.tile([128, NI * D], BF16, name="r_all")
            stgr = sb.tile([128, STG], BF16, name="stgr")
            with tc.For_i(0, 8) as wv:
                # stgr[p, z] = mcat[p + 512w + z], z in [0, 4480)
                nc.sync.dma_start(
                    stgr[:],
                    AP(mcat[:].tensor, wv * 512, [[1, 128], [1, STG]]))
                for qb in range(4):
                    r_ps = ps.tile([128, 512], F32, name=f"r_ps{qb}")
                    for bc in range(32):
                        nc.tensor.matmul(
                            r_ps[:],
                            qt_all[:, NS * bc + 128 * qb:
                                   NS * bc + 128 * qb + 128],
                            stgr[:, 128 * bc:128 * bc + 512],
                            start=(bc == 0), stop=(bc == 31),
                        )
                    dst = bass.ds(wv * 512 + D * qb, 512)
                    if qb % 2 == 0:
                        nc.vector.tensor_copy(r_all[:, dst], r_ps[:])
                    else:
                        nc.scalar.copy(r_all[:, dst], r_ps[:])
            nc.sync.dma_start(
                r_out.rearrange("(qb p) n -> p qb n", p=128),
                r_all[:].rearrange("p (qb n) -> p qb n", qb=NI))
    _split_waits(nc)
    return nc


_warm = {"nc": None, "err": None}


def _warm_jax():
    try:
        import jax
        jax.devices()
    except Exception:
        pass


def _warm_build():
    try:
        _warm["nc"] = _build()
    except Exception as e:  # rebuild inline in kernel() on failure
        _warm["err"] = e


_thr_jax = threading.Thread(target=_warm_jax, daemon=True)
_thr_jax.start()
_thr_build = threading.Thread(target=_warm_build, daemon=True)
_thr_build.start()


def _run(nc, in_maps):
    import time
    t0 = time.time()
    res = run_bass_kernel_spmd(nc, in_maps, core_ids=list(range(NCORES)))
    LAST_EXEC_NS.append(int((time.time() - t0) * 1e9))
    return res.results


def kernel(keys, values, query_keys):
    keys = np.asarray(keys)
    values = np.asarray(values)
    query_keys = np.asarray(query_keys)

    in_maps = []
    for c in range(NCORES):
        sl = slice(NS * c, NS * (c + 1))
        in_maps.append({
            "k_in": np.ascontiguousarray(keys[sl].astype(NPBF16)),
            "v_in": np.ascontiguousarray(values[sl].astype(NPBF16)),
            "q_in": np.ascontiguousarray(query_keys[sl].astype(NPBF16)),
        })

    _thr_jax.join(timeout=300)
    _thr_build.join(timeout=300)
    nc = _warm["nc"] if _warm["nc"] is not None else _build()

    # The terminal occasionally reports NRT_EXEC_UNIT_UNRECOVERABLE and
    # needs ~1-2 min to self-heal; retry rather than fail the whole call.
    outs = None
    for attempt, delay in ((0, 0), (1, 30), (2, 120)):
        try:
            if attempt:
                import time
                time.sleep(delay)
                try:
                    import jax
                    jax.clear_caches()
                except Exception:
                    pass
                nc = _build()
            outs = _run(nc, in_maps)
            break
        except Exception:
            if attempt == 2:
                raise
    res = np.empty((D, D), np.float32)
    for c in range(NCORES):
        res[NS * c:NS * (c + 1), :] = outs[c]["r_out"]  # bf16 -> f32 cast
    return res
